# revision 6
# baseline (speedup 1.0000x reference)
"""Causal self-attention (B=1, T=4096, C=768, H=12) on 8 Trainium2 NeuronCores.

Sharding: tensor-parallel over heads. 16 head-slots across 8 cores (2 slots
per core); 12 real heads + 4 dummy slots with zeroed weights. Each core:
  1. transposes x -> x^T on the PE array (needed as matmul contraction layout)
  2. projects Q^T, K^T, V^T for its 2 head-slots (full T)
  3. runs causal flash-style attention fully on-chip in the transposed
     layout: S^T[k,q] = K^T.T @ Q^T per 128-wide k-block, P^T = exp(S^T/8)
     (scores are small enough that max-subtraction is unnecessary), causal
     masking via static triangular masks on the diagonal blocks only, and
     y^T accumulated in PSUM with an extra all-ones column in V providing
     the softmax denominator in row 64.
  4. normalizes y^T and computes a partial output projection with its
     128-row slice of w_proj.
The host sums the 8 partial [4096, 768] outputs -- no device collectives.

Causal load within a core is balanced by processing q-blocks in pairs
(i, 15-i) of 256 rows: each pair touches exactly 34 k-blocks.
"""

import sys

sys.path.insert(0, "/opt/trn_rl_repo")

import numpy as np

T = 4096
C = 768
H = 12
HD = 64
N_CORES = 8
SLOTS = 2
TS = 512  # t-slice for x load/transpose/projection
NTS = T // TS  # 8
QB = 256  # q-block rows
NQB = T // QB  # 16
KB = 128  # k-block rows
NKB = T // KB  # 32
NPAIR = NQB // 2  # 8 causal-balanced pairs (i, 15-i)

_CACHE = {}


def _paired_col(b256: int) -> int:
    """Column offset of 256-row q-block b256 in the paired SBUF layout.

    Pair p = min(b, 15-b) occupies cols [512p, 512p+512): side A (b < 8)
    at +0, side B (b >= 8) at +256.
    """
    p = min(b256, NQB - 1 - b256)
    side = 1 if b256 >= NQB // 2 else 0
    return 2 * QB * p + QB * side


def _build_nc():
    import concourse.bacc as bacc
    import concourse.tile as tile
    import concourse.mybir as mybir
    from concourse.masks import make_identity
    from contextlib import ExitStack

    F32 = mybir.dt.float32
    F32R = mybir.dt.float32r
    EXP = mybir.ActivationFunctionType.Exp

    nc = bacc.Bacc(
        "TRN2",
        target_bir_lowering=False,
        debug=False,
        enable_asserts=True,
        num_devices=N_CORES,
    )
    x_d = nc.dram_tensor("x", [T, C], F32, kind="ExternalInput")
    wa_d = nc.dram_tensor("wa", [C, 3 * SLOTS * HD], F32R, kind="ExternalInput")
    wp_d = nc.dram_tensor("wp", [SLOTS * HD, C], F32R, kind="ExternalInput")
    out_d = nc.dram_tensor("out", [T, C], F32, kind="ExternalOutput")

    with ExitStack() as ctx:
        tc = ctx.enter_context(tile.TileContext(nc))
        singles = ctx.enter_context(tc.tile_pool(name="singles", bufs=1))
        xpool = ctx.enter_context(tc.tile_pool(name="xpool", bufs=6))
        xtpool = ctx.enter_context(tc.tile_pool(name="xtpool", bufs=12))
        ptpool = ctx.enter_context(tc.tile_pool(name="ptpool", bufs=4))
        rpool = ctx.enter_context(tc.tile_pool(name="rpool", bufs=4))
        opool = ctx.enter_context(tc.tile_pool(name="opool", bufs=4))
        ps = ctx.enter_context(tc.tile_pool(name="ps", bufs=8, space="PSUM"))

        # ---- persistent SBUF tensors ----
        qt = singles.tile([128, T], F32R)  # Q^T, paired column layout
        kt = singles.tile([128, T], F32R)  # K^T, natural column layout
        yt_all = singles.tile([128, T], F32R)  # normalized y^T, paired layout
        v1 = singles.tile([128, NKB, SLOTS, HD + 1], F32R)  # V blocks + ones col
        wa_sb = []
        for i in range(6):
            wa_c = singles.tile([128, 3 * SLOTS * HD], F32R, name=f"wa_c{i}")
            wa_sb.append(wa_c)
        wp_sb = singles.tile([SLOTS * HD, C], F32R)
        ident = singles.tile([128, 128], F32)
        ones64 = singles.tile([1, HD], F32R)
        maskf = singles.tile([128, 3 * KB], F32)  # [:,128:384]=M0, [:,0:256]=M1

        make_identity(nc, ident)
        ones_f32 = singles.tile([128, NKB * SLOTS], F32)
        nc.gpsimd.memset(ones_f32, 1.0)
        nc.vector.tensor_copy(out=ones64, in_=ones_f32[0:1, 0:HD])
        nc.vector.tensor_copy(
            out=v1[:, :, :, HD : HD + 1],
            in_=ones_f32.rearrange("p (a b) -> p a b", a=NKB).unsqueeze(3),
        )

        # maskf[k, c] = 1 if c >= k + 128 else 0
        nc.gpsimd.memset(maskf, 0.0)
        nc.gpsimd.affine_select(
            out=maskf,
            in_=maskf,
            compare_op=mybir.AluOpType.is_gt,
            fill=1.0,
            base=KB,
            channel_multiplier=1,
            pattern=[[-1, 3 * KB]],
        )

        for i in range(6):
            nc.sync.dma_start(out=wa_sb[i], in_=wa_d.ap()[i * 128 : (i + 1) * 128, :])
        nc.sync.dma_start(out=wp_sb, in_=wp_d.ap())

        # ---- phase A/B: x -> x^T -> Q^T/K^T/V per t-slice ----
        for ts in range(NTS):
            xs = []
            for tb in range(4):
                r0 = ts * TS + tb * 128
                xst = xpool.tile([128, C], F32, name="xs", tag="xs")
                nc.sync.dma_start(out=xst, in_=x_d.ap()[r0 : r0 + 128, :])
                xs.append(xst)
            xts = []
            for ic in range(6):
                xtt = xtpool.tile([128, TS], F32R, name="xt", tag="xt")
                for tb in range(4):
                    tps = ps.tile([128, 128], F32, name="tps", tag="ps")
                    nc.tensor.transpose(
                        tps, xs[tb][:, ic * 128 : (ic + 1) * 128], ident
                    )
                    nc.vector.tensor_copy(
                        out=xtt[:, tb * 128 : (tb + 1) * 128], in_=tps
                    )
                xts.append(xtt)
            for p in range(3):
                pp = ps.tile([128, TS], F32, name="pp", tag="ps")
                for ic in range(6):
                    nc.tensor.matmul(
                        pp,
                        lhsT=wa_sb[ic][:, p * 128 : (p + 1) * 128],
                        rhs=xts[ic],
                        start=(ic == 0),
                        stop=(ic == 5),
                    )
                if p == 0:
                    for half in range(2):
                        col = _paired_col(2 * ts + half)
                        nc.vector.tensor_copy(
                            out=qt[:, col : col + QB],
                            in_=pp[:, half * QB : (half + 1) * QB],
                        )
                elif p == 1:
                    nc.vector.tensor_copy(out=kt[:, ts * TS : (ts + 1) * TS], in_=pp)
                else:
                    vt = rpool.tile([128, TS], F32, name="vt", tag="vt")
                    nc.vector.tensor_copy(out=vt, in_=pp)
                    for sub in range(4):
                        kb = 4 * ts + sub
                        vps = ps.tile([128, 128], F32, name="vps", tag="ps")
                        nc.tensor.transpose(
                            vps, vt[:, sub * 128 : (sub + 1) * 128], ident
                        )
                        for s in range(SLOTS):
                            nc.vector.tensor_copy(
                                out=v1[:, kb, s, 0:HD],
                                in_=vps[:, s * HD : (s + 1) * HD],
                            )

        # ---- phase C: attention per (slot, pair) ----
        scale = 1.0 / float(np.sqrt(HD))
        for s in range(SLOTS):
            r0, r1 = s * HD, (s + 1) * HD
            for i in reversed(range(NPAIR)):
                qcol = 2 * QB * i
                n_shared = 2 * i + 2  # k-blocks needed by side A (block i)
                n_total = NKB - 2 * i  # k-blocks needed by side B (block 15-i)
                diag_b0 = NKB - 2 - 2 * i  # first diagonal k-block of side B
                ytA = ps.tile([HD + 1, QB], F32, name="ytA", tag="ps")
                ytB = ps.tile([HD + 1, QB], F32, name="ytB", tag="ps")
                for kb in range(n_total):
                    shared = kb < n_shared
                    width = 2 * QB if shared else QB
                    qoff = qcol if shared else qcol + QB
                    st = ps.tile([128, width], F32, name="st", tag="ps")
                    nc.tensor.matmul(
                        st,
                        lhsT=kt[r0:r1, kb * KB : (kb + 1) * KB],
                        rhs=qt[r0:r1, qoff : qoff + width],
                        start=True,
                        stop=True,
                    )
                    pt = ptpool.tile([128, 2 * QB], F32R, name="pt", tag="pt")
                    nc.scalar.activation(
                        out=pt[:, 0:width], in_=st, func=EXP, scale=scale
                    )
                    # causal masks on diagonal blocks
                    if shared and kb == 2 * i:
                        nc.vector.tensor_mul(
                            out=pt[:, 0:QB], in0=pt[:, 0:QB], in1=maskf[:, KB : KB + QB]
                        )
                    if shared and kb == 2 * i + 1:
                        nc.vector.tensor_mul(
                            out=pt[:, 0:QB], in0=pt[:, 0:QB], in1=maskf[:, 0:QB]
                        )
                    if not shared and kb == diag_b0:
                        nc.vector.tensor_mul(
                            out=pt[:, 0:QB], in0=pt[:, 0:QB], in1=maskf[:, KB : KB + QB]
                        )
                    if not shared and kb == diag_b0 + 1:
                        nc.vector.tensor_mul(
                            out=pt[:, 0:QB], in0=pt[:, 0:QB], in1=maskf[:, 0:QB]
                        )
                    vblk = v1[:, kb, s, :]
                    if shared:
                        nc.tensor.matmul(
                            ytA,
                            lhsT=vblk,
                            rhs=pt[:, 0:QB],
                            start=(kb == 0),
                            stop=(kb == n_shared - 1),
                        )
                        nc.tensor.matmul(
                            ytB,
                            lhsT=vblk,
                            rhs=pt[:, QB : 2 * QB],
                            start=(kb == 0),
                            stop=False,
                        )
                    else:
                        nc.tensor.matmul(
                            ytB,
                            lhsT=vblk,
                            rhs=pt[:, 0:QB],
                            start=False,
                            stop=(kb == n_total - 1),
                        )
                # softmax normalization: row HD holds the denominators
                for ytX, col in ((ytA, qcol), (ytB, qcol + QB)):
                    r_tmp = rpool.tile([1, QB], F32, name="r_tmp", tag="r_tmp")
                    nc.vector.reciprocal(out=r_tmp, in_=ytX[HD : HD + 1, :])
                    r_sb = rpool.tile([1, QB], F32R, name="r_sb", tag="r_sb")
                    nc.vector.tensor_copy(out=r_sb, in_=r_tmp)
                    bc = ps.tile([HD, QB], F32, name="bc", tag="ps")
                    nc.tensor.matmul(
                        bc,
                        lhsT=ones64,
                        rhs=r_sb,
                        start=True,
                        stop=True,
                    )
                    bc_sb = rpool.tile([HD, QB], F32, name="bc_sb", tag="bc_sb")
                    nc.vector.tensor_copy(out=bc_sb, in_=bc)
                    nc.vector.tensor_mul(
                        out=yt_all[r0:r1, col : col + QB],
                        in0=ytX[0:HD, :],
                        in1=bc_sb,
                    )

        # ---- phase D: partial output projection ----
        for tb in range(NKB):  # 32 blocks of 128 rows
            b256, half = tb // 2, tb % 2
            col = _paired_col(b256) + 128 * half
            for nh in range(2):
                po = ps.tile([128, C // 2], F32, name="po", tag="ps")
                nc.tensor.matmul(
                    po,
                    lhsT=yt_all[:, col : col + 128],
                    rhs=wp_sb[:, nh * 384 : (nh + 1) * 384],
                    start=True,
                    stop=True,
                )
                osb = opool.tile([128, C // 2], F32, name="osb", tag="osb")
                nc.vector.tensor_copy(out=osb, in_=po)
                nc.sync.dma_start(
                    out=out_d.ap()[
                        tb * 128 : (tb + 1) * 128, nh * 384 : (nh + 1) * 384
                    ],
                    in_=osb,
                )

    nc.compile()
    return nc


def _get_nc():
    if "nc" not in _CACHE:
        _CACHE["nc"] = _build_nc()
    return _CACHE["nc"]


def _core_inputs(x, w_attn, w_proj):
    """Build per-core input dicts (head-slot weight slices)."""
    x = np.ascontiguousarray(x.reshape(T, C), dtype=np.float32)
    w_attn = np.asarray(w_attn, dtype=np.float32)
    w_proj = np.asarray(w_proj, dtype=np.float32)
    in_maps = []
    for c in range(N_CORES):
        heads = [c, 8 + c if c < 4 else None]
        wa = np.zeros((C, 3, SLOTS, HD), dtype=np.float32)
        wp = np.zeros((SLOTS * HD, C), dtype=np.float32)
        for s, h in enumerate(heads):
            if h is None:
                continue
            for p in range(3):
                wa[:, p, s, :] = w_attn[:, p * C + h * HD : p * C + (h + 1) * HD]
            wp[s * HD : (s + 1) * HD, :] = w_proj[h * HD : (h + 1) * HD, :]
        in_maps.append(
            {"x": x, "wa": np.ascontiguousarray(wa.reshape(C, 3 * SLOTS * HD)), "wp": wp}
        )
    return in_maps


def kernel(x, w_attn, w_proj):
    from concourse.bass_utils import run_bass_kernel_spmd

    nc = _get_nc()
    in_maps = _core_inputs(np.asarray(x), np.asarray(w_attn), np.asarray(w_proj))
    res = run_bass_kernel_spmd(nc, in_maps, core_ids=list(range(N_CORES)), trace=False)
    out = np.zeros((T, C), dtype=np.float32)
    for c in range(N_CORES):
        out += res.results[c]["out"]
    return out.reshape(1, T, C)


# revision 26
# speedup vs baseline: 14454.9060x; 14454.9060x over previous
"""Causal self-attention (B=1, T=4096, C=768, H=12) on 8 Trainium2 NeuronCores.

Sharding: tensor-parallel over heads. 16 head-slots across 8 cores (2 slots
per core); 12 real heads + 4 dummy slots with zeroed weights. Each core:
  1. transposes x -> x^T on the PE array (needed as matmul contraction layout)
  2. projects Q^T, K^T, V^T for its 2 head-slots (full T)
  3. runs causal flash-style attention fully on-chip in the transposed
     layout: S^T[k,q] = K^T.T @ Q^T per 128-wide k-block, P^T = exp(S^T/8)
     (scores are small enough that max-subtraction is unnecessary), causal
     masking via static triangular masks on the diagonal blocks only, and
     y^T accumulated in PSUM with an extra all-ones column in V providing
     the softmax denominator in row 64.
  4. normalizes y^T and computes a partial output projection with its
     128-row slice of w_proj.
The host sums the 8 partial [4096, 768] outputs -- no device collectives.

Causal load within a core is balanced by processing q-blocks in pairs
(i, 15-i) of 256 rows: each pair touches exactly 34 k-blocks.
"""

import sys

sys.path.insert(0, "/opt/trn_rl_repo")

import numpy as np

T = 4096
C = 768
H = 12
HD = 64
N_CORES = 8
SLOTS = 2
TS = 512  # t-slice for x load/transpose/projection
NTS = T // TS  # 8
QB = 256  # q-block rows
NQB = T // QB  # 16
KB = 128  # k-block rows
NKB = T // KB  # 32
NPAIR = NQB // 2  # 8 causal-balanced pairs (i, 15-i)

_CACHE = {}


def _paired_col(b256: int) -> int:
    """Column offset of 256-row q-block b256 in the paired SBUF layout.

    Pair p = min(b, 15-b) occupies cols [512p, 512p+512): side A (b < 8)
    at +0, side B (b >= 8) at +256.
    """
    p = min(b256, NQB - 1 - b256)
    side = 1 if b256 >= NQB // 2 else 0
    return 2 * QB * p + QB * side


def _build_nc():
    import concourse.bacc as bacc
    import concourse.tile as tile
    import concourse.mybir as mybir
    from concourse.masks import make_identity
    from contextlib import ExitStack

    F32 = mybir.dt.float32
    F32R = mybir.dt.float32r
    EXP = mybir.ActivationFunctionType.Exp

    nc = bacc.Bacc(
        "TRN2",
        target_bir_lowering=False,
        debug=False,
        enable_asserts=True,
        num_devices=N_CORES,
    )
    x_d = nc.dram_tensor("x", [T, C], F32R, kind="ExternalInput")
    wa_d = nc.dram_tensor("wa", [C, 3 * SLOTS * HD], F32R, kind="ExternalInput")
    wp_d = nc.dram_tensor("wp", [SLOTS * HD, C], F32R, kind="ExternalInput")
    out_d = nc.dram_tensor("out", [T, C], F32, kind="ExternalOutput")

    with ExitStack() as ctx:
        tc = ctx.enter_context(tile.TileContext(nc))
        singles = ctx.enter_context(tc.tile_pool(name="singles", bufs=1))
        xpool = ctx.enter_context(tc.tile_pool(name="xpool", bufs=8))
        xtpool = ctx.enter_context(tc.tile_pool(name="xtpool", bufs=8))
        ptpool = ctx.enter_context(tc.tile_pool(name="ptpool", bufs=8))
        rpool = ctx.enter_context(tc.tile_pool(name="rpool", bufs=4))
        opool = ctx.enter_context(tc.tile_pool(name="opool", bufs=4))
        ps = ctx.enter_context(tc.tile_pool(name="ps", bufs=2, space="PSUM"))
        ps_st = ctx.enter_context(tc.tile_pool(name="ps_st", bufs=2, space="PSUM"))
        ps_yt = ctx.enter_context(tc.tile_pool(name="ps_yt", bufs=2, space="PSUM"))

        # ---- persistent SBUF tensors ----
        qt = singles.tile([128, T], F32R)  # Q^T, paired column layout
        kt = singles.tile([128, T], F32R)  # K^T, natural column layout
        yt_all = singles.tile([128, T], F32R)  # normalized y^T, paired layout
        v1 = singles.tile([128, NKB, SLOTS, HD + 1], F32R)  # V blocks + ones col
        wa_sb = []
        for i in range(6):
            wa_c = singles.tile([128, 3 * SLOTS * HD], F32R, name=f"wa_c{i}")
            wa_sb.append(wa_c)
        wp_sb = singles.tile([SLOTS * HD, C], F32R)
        ident = singles.tile([128, 128], F32R)
        ones64 = singles.tile([1, HD], F32R)
        maskf = singles.tile([128, 3 * KB], F32)  # [:,128:384]=M0, [:,0:256]=M1

        ident_f32 = singles.tile([128, 128], F32)
        make_identity(nc, ident_f32)
        nc.vector.tensor_copy(out=ident, in_=ident_f32)
        ones_f32 = singles.tile([128, NKB * SLOTS], F32)
        nc.gpsimd.memset(ones_f32, 1.0)
        nc.vector.tensor_copy(out=ones64, in_=ones_f32[0:1, 0:HD])
        nc.vector.tensor_copy(
            out=v1[:, :, :, HD : HD + 1],
            in_=ones_f32.rearrange("p (a b) -> p a b", a=NKB).unsqueeze(3),
        )

        # maskf[k, c] = 1 if c >= k + 128 else 0
        nc.gpsimd.memset(maskf, 0.0)
        nc.gpsimd.affine_select(
            out=maskf,
            in_=maskf,
            compare_op=mybir.AluOpType.is_gt,
            fill=1.0,
            base=KB,
            channel_multiplier=1,
            pattern=[[-1, 3 * KB]],
        )

        for i in range(6):
            nc.gpsimd.dma_start(out=wa_sb[i], in_=wa_d.ap()[i * 128 : (i + 1) * 128, :])
        nc.gpsimd.dma_start(out=wp_sb, in_=wp_d.ap())

        # ---- phase A/B: x -> x^T -> Q^T/K^T/V per t-slice ----
        for ts in range(NTS):
            xs = []
            for tb in range(4):
                r0 = ts * TS + tb * 128
                xst = xpool.tile([128, C], F32R, name="xs", tag="xs")
                nc.sync.dma_start(out=xst, in_=x_d.ap()[r0 : r0 + 128, :])
                xs.append(xst)
            xts = []
            for ic in range(6):
                xtt = xtpool.tile([128, TS], F32R, name="xt", tag="xt")
                tps = ps.tile([128, TS], F32R, name="tps", tag="ps")
                for tb in range(4):
                    nc.tensor.transpose(
                        tps[:, tb * 128 : (tb + 1) * 128],
                        xs[tb][:, ic * 128 : (ic + 1) * 128],
                        ident,
                    )
                nc.vector.tensor_copy(out=xtt, in_=tps)
                xts.append(xtt)
            for p in range(3):
                pp = ps.tile([128, TS], F32, name="pp", tag="ps")
                for ic in range(6):
                    nc.tensor.matmul(
                        pp,
                        lhsT=wa_sb[ic][:, p * 128 : (p + 1) * 128],
                        rhs=xts[ic],
                        start=(ic == 0),
                        stop=(ic == 5),
                    )
                if p == 0:
                    for half in range(2):
                        col = _paired_col(2 * ts + half)
                        nc.vector.tensor_copy(
                            out=qt[:, col : col + QB],
                            in_=pp[:, half * QB : (half + 1) * QB],
                        )
                elif p == 1:
                    nc.vector.tensor_copy(out=kt[:, ts * TS : (ts + 1) * TS], in_=pp)
                else:
                    vt = rpool.tile([128, TS], F32R, name="vt", tag="vt", bufs=3)
                    nc.vector.tensor_copy(out=vt, in_=pp)
                    for sub in range(4):
                        kb = 4 * ts + sub
                        vps = ps.tile([128, 128], F32R, name="vps", tag="ps")
                        nc.tensor.transpose(
                            vps,
                            vt[:, sub * 128 : (sub + 1) * 128],
                            ident,
                        )
                        nc.vector.tensor_copy(
                            out=v1[:, kb, :, 0:HD],
                            in_=vps.rearrange("p (s d) -> p s d", s=SLOTS),
                        )

        # ---- phase C: attention, slots interleaved per pair; phase D
        # (partial projection) emitted as soon as a pair completes ----
        scale = 1.0 / float(np.sqrt(HD))

        def emit_proj(tb):
            b256, half = tb // 2, tb % 2
            col = _paired_col(b256) + 128 * half
            po = ps_st.tile([128, C], F32, name="po", tag="st")
            for c0, c1 in ((0, 512), (512, 768)):  # bank-aligned splits
                nc.tensor.matmul(
                    po[:, c0:c1],
                    lhsT=yt_all[:, col : col + 128],
                    rhs=wp_sb[:, c0:c1],
                    start=True,
                    stop=True,
                )
            osb = opool.tile([128, C], F32, name="osb", tag="osb")
            nc.vector.tensor_copy(out=osb, in_=po)
            nc.sync.dma_start(
                out=out_d.ap()[tb * 128 : (tb + 1) * 128, :], in_=osb
            )

        import collections
        work_q = collections.deque()  # deferred closures, drained between groups

        def emit_norm(ytsb, r0, r1, col):
            def go():
                r_sb = rpool.tile([1, 2 * QB], F32R, name="r_sb", tag="r_sb", bufs=8)
                with nc.allow_low_precision(reason="fp32r softmax denom"):
                    nc.vector.reciprocal(out=r_sb, in_=ytsb[HD : HD + 1, :])
                bc = ps.tile([HD, 2 * QB], F32, name="bc", tag="ps")
                nc.tensor.matmul(
                    bc,
                    lhsT=ones64,
                    rhs=r_sb,
                    start=True,
                    stop=True,
                )
                nc.vector.tensor_mul(
                    out=yt_all[r0:r1, col : col + 2 * QB],
                    in0=ytsb[0:HD, :],
                    in1=bc,
                )
            return go

        for i in reversed(range(NPAIR)):
            for s in range(SLOTS):
                r0, r1 = s * HD, (s + 1) * HD
                qcol = 2 * QB * i
                n_shared = 2 * i + 2  # k-blocks needed by side A (block i)
                n_total = NKB - 2 * i  # k-blocks needed by side B (block 15-i)
                diag_b0 = NKB - 2 - 2 * i  # first diagonal k-block of side B
                yt = ps_yt.tile([HD + 1, 2 * QB], F32, name="yt", tag="yt")
                # k-blocks in groups sharing one wide PSUM score tile:
                # shared region (A+B, q-width 512) pairs 2 k-blocks; solo
                # region (B only, q-width 256) packs 4. One exp per group.
                groups = [list(range(g, g + 2)) for g in range(0, n_shared, 2)]
                kb0 = n_shared
                while kb0 < n_total:
                    n = min(4, n_total - kb0)
                    groups.append(list(range(kb0, kb0 + n)))
                    kb0 += n
                def emit_s(grp):
                    shared = grp[0] < n_shared
                    w = 2 * QB if shared else QB
                    qoff = qcol if shared else qcol + QB
                    gw = w * len(grp)
                    st = ps_st.tile([128, 4 * QB], F32, name="st", tag="st")
                    for j, kb in enumerate(grp):
                        nc.tensor.matmul(
                            st[:, j * w : (j + 1) * w],
                            lhsT=kt[r0:r1, kb * KB : (kb + 1) * KB],
                            rhs=qt[r0:r1, qoff : qoff + w],
                            start=True,
                            stop=True,
                        )
                    pt = ptpool.tile([128, 4 * QB], F32R, name="pt", tag="pt")
                    nc.scalar.activation(
                        out=pt[:, 0:gw], in_=st[:, 0:gw], func=EXP, scale=scale
                    )
                    for j, kb in enumerate(grp):
                        pA = pt[:, j * w : j * w + QB]
                        if kb == 2 * i or kb == diag_b0:
                            nc.vector.tensor_mul(
                                out=pA, in0=pA, in1=maskf[:, KB : KB + QB]
                            )
                        if kb == 2 * i + 1 or kb == diag_b0 + 1:
                            nc.vector.tensor_mul(
                                out=pA, in0=pA, in1=maskf[:, 0:QB]
                            )
                    return pt, w

                def emit_pv(grp, pt, w):
                    # one PSUM accumulation group spans the whole pair:
                    # started once at kb==0 (full width), A columns simply
                    # stop being written after the shared region ends,
                    # stop flags on the final solo matmul
                    shared = grp[0] < n_shared
                    for j, kb in enumerate(grp):
                        vblk = v1[:, kb, s, :]
                        if shared:
                            nc.tensor.matmul(
                                yt,
                                lhsT=vblk,
                                rhs=pt[:, j * w : (j + 1) * w],
                                start=(kb == 0),
                                stop=False,
                                skip_group_check=True,
                            )
                        else:
                            nc.tensor.matmul(
                                yt[:, QB : 2 * QB],
                                lhsT=vblk,
                                rhs=pt[:, j * w : (j + 1) * w],
                                start=False,
                                stop=(kb == n_total - 1),
                                skip_group_check=True,
                            )

                pending = None
                for grp in groups:
                    cur = (grp, *emit_s(grp))
                    if pending is not None:
                        emit_pv(*pending)
                    pending = cur
                    if work_q:
                        work_q.popleft()()
                emit_pv(*pending)
                # free the yt PSUM slot immediately; queue the rest of
                # the normalization to drain between later matmul groups
                ytsb = rpool.tile([HD + 1, 2 * QB], F32, name="ytsb", tag="ytsb", bufs=6)
                nc.vector.tensor_copy(out=ytsb, in_=yt)
                work_q.append(emit_norm(ytsb, r0, r1, qcol))
            for tb in (2 * i, 2 * i + 1, NKB - 2 - 2 * i, NKB - 1 - 2 * i):
                work_q.append(lambda tb=tb: emit_proj(tb))
        while work_q:
            work_q.popleft()()


    nc.compile()
    return nc


def _get_nc():
    if "nc" not in _CACHE:
        _CACHE["nc"] = _build_nc()
    return _CACHE["nc"]


def _core_inputs(x, w_attn, w_proj):
    """Build per-core input dicts (head-slot weight slices)."""
    x = np.ascontiguousarray(x.reshape(T, C), dtype=np.float32)
    w_attn = np.asarray(w_attn, dtype=np.float32)
    w_proj = np.asarray(w_proj, dtype=np.float32)
    in_maps = []
    for c in range(N_CORES):
        heads = [c, 8 + c if c < 4 else None]
        wa = np.zeros((C, 3, SLOTS, HD), dtype=np.float32)
        wp = np.zeros((SLOTS * HD, C), dtype=np.float32)
        for s, h in enumerate(heads):
            if h is None:
                continue
            for p in range(3):
                wa[:, p, s, :] = w_attn[:, p * C + h * HD : p * C + (h + 1) * HD]
            wp[s * HD : (s + 1) * HD, :] = w_proj[h * HD : (h + 1) * HD, :]
        in_maps.append(
            {"x": x, "wa": np.ascontiguousarray(wa.reshape(C, 3 * SLOTS * HD)), "wp": wp}
        )
    return in_maps


def _get_runner():
    """Build the shard_map'd PJRT executable once and reuse it across calls.

    Mirrors bass2jax.run_bass_via_pjrt's multi-core path, but caches the
    jitted callable so repeat kernel() calls skip re-trace/re-compile.
    """
    if "runner" in _CACHE:
        return _CACHE["runner"]
    import jax
    import concourse.mybir as mybir
    from concourse import bass2jax
    from jax.experimental.shard_map import shard_map
    from jax.sharding import Mesh, PartitionSpec

    nc = _get_nc()
    bass2jax.install_neuronx_cc_hook()

    in_names, out_names, out_avals, zero_outs = [], [], [], []
    for alloc in nc.m.functions[0].allocations:
        if not isinstance(alloc, mybir.MemoryLocationSet):
            continue
        name = alloc.memorylocations[0].name
        if alloc.kind == "ExternalInput":
            if nc.partition_id_tensor and name == nc.partition_id_tensor.name:
                continue
            in_names.append(name)
        elif alloc.kind == "ExternalOutput":
            shape = tuple(alloc.tensor_shape)
            dtype = mybir.dt.np(alloc.dtype)
            out_names.append(name)
            out_avals.append(jax.core.ShapedArray(shape, dtype))
            zero_outs.append(np.zeros(shape, dtype))
    n_params = len(in_names)
    all_in_names = in_names + out_names
    if nc.partition_id_tensor:
        all_in_names = all_in_names + [nc.partition_id_tensor.name]

    def _body(*args):
        operands = list(args)
        if nc.partition_id_tensor:
            operands.append(bass2jax.partition_id_tensor())
        outs = bass2jax._bass_exec_p.bind(
            *operands,
            out_avals=tuple(out_avals),
            in_names=tuple(all_in_names),
            out_names=tuple(out_names),
            lowering_input_output_aliases=(),
            sim_require_finite=True,
            sim_require_nnan=True,
            nc=nc,
        )
        return tuple(outs)

    devices = jax.devices()[:N_CORES]
    mesh = Mesh(np.asarray(devices), ("core",))
    n_out = len(out_names)
    donate = tuple(range(n_params, n_params + n_out))
    sharded = jax.jit(
        shard_map(
            _body,
            mesh=mesh,
            in_specs=(PartitionSpec("core"),) * (n_params + n_out),
            out_specs=(PartitionSpec("core"),) * n_out,
            check_rep=False,
        ),
        donate_argnums=donate,
        keep_unused=True,
    )

    def run(in_maps):
        concat_in = [
            np.concatenate([in_maps[c][name] for c in range(N_CORES)], axis=0)
            for name in in_names
        ]
        concat_zeros = [
            np.zeros((N_CORES * z.shape[0], *z.shape[1:]), z.dtype)
            for z in zero_outs
        ]
        out_arrs = sharded(*concat_in, *concat_zeros)
        return [
            {
                name: np.asarray(out_arrs[i]).reshape(
                    N_CORES, *out_avals[i].shape
                )[c]
                for i, name in enumerate(out_names)
            }
            for c in range(N_CORES)
        ]

    _CACHE["runner"] = run
    return run


def kernel(x, w_attn, w_proj):
    run = _get_runner()
    in_maps = _core_inputs(np.asarray(x), np.asarray(w_attn), np.asarray(w_proj))
    results = run(in_maps)
    out = np.zeros((T, C), dtype=np.float32)
    for c in range(N_CORES):
        out += results[c]["out"]
    return out.reshape(1, T, C)
